# revision 1
# baseline (speedup 1.0000x reference)
"""AdaptiveBarlowTwinsLoss on 8 TRN2 NeuronCores.

Math: with O = head_outputs reshaped (N, H*dh), standardized O~ = (O-mu)/(sigma+eps),
the loss only needs the 120 upper-triangular head-pair blocks of C = O~^T O~ / N.
Writing G = O^T O (raw gram), C[id, je] = G[id,je]*r[id]*r[je] - q[id]*q[je]
with r = 1/(sqrt(N)(sigma+eps)), q = sqrt(N)*mu*r.

Distribution: data-parallel over tokens. Each core computes the raw bf16 gram
blocks over its 2048-token shard (fp8 DoubleRow matmuls, fp32 PSUM accum) and
a bf16 ReduceScatter sums them across cores while scattering 15 pairs to each
core — the RS chain overlaps the gram since raw blocks need no statistics.
Statistics (fp32 S1/S2 column sums, partition-reduced by a ones-matmul) are
AllReduced (16KB) in a collective-queue slot between the ReduceScatters. Post-RS, each core standardizes only its own 15 blocks:
per-core 0/1 selector input tensors turn the core-specific (i,j) head choices
into tiny on-device matmuls that gather r/q into per-slot scale tiles (keeps
the SPMD program identical across cores). Finally ||C_ij - I||_F^2 per pair;
the host applies the (16,16) softplus pair weights and averages.

Pair p (lexicographic (i,j), i<j) is assigned to core p % 8, slot p // 8.

Engine discipline (scheduled FIFO per engine — stats-gated work must never
precede progressive work on the same engine):
  PE:  S1/S2 reduce, pair-seg gens (+ late: transposes, selector mms)
  ACT: fp8 casts, squares, PSUM->SBUF copies, sqrt, final squares
  DVE: S1/S2 accumulate, stats math, selector-tile glue, post-RS
  GpSimd: constants, bounce DMAs, collectives
"""

import math
import sys

sys.path.insert(0, "/opt/trn_rl_repo")

import numpy as np

import concourse.bass as bass
import concourse.tile as tile
from concourse import bacc, mybir
from concourse.bass_utils import run_bass_kernel_spmd
from concourse.masks import make_identity

B, T, H, DH = 8, 2048, 16, 128
N = B * T                      # 16384 tokens
F = H * DH                     # 2048 features
NC = 8                         # cores
NS = N // NC                   # 2048 tokens per core
KCH = NS // 128                # 16 contraction chunks of 128 tokens
ALPHA, BETA, TAU, EPS = 0.929, 15.99, 0.0, 1e-8

PAIRS = [(i, j) for i in range(H) for j in range(i + 1, H)]   # 120, lex order
NSLOT = len(PAIRS) // 8                                       # 15 slots per core
RS_SLOTS = [(0, 5), (5, 10), (10, 15)]                        # 3 ReduceScatter calls

F32 = mybir.dt.float32
BF16 = mybir.dt.bfloat16
FP8 = mybir.dt.float8e4


def _segments():
    """Row segments (i, j0, nb, pbase): same i, consecutive j, nb<=8."""
    segs = []
    p0 = 0
    for i in range(H):
        j = i + 1
        while j < H:
            nb = min(8, H - j)
            segs.append((i, j, nb, p0))
            p0 += nb
            j += nb
    assert p0 == len(PAIRS)
    return segs


def build():
    nc = bacc.Bacc("TRN2", target_bir_lowering=False, debug=False, num_devices=NC)

    x = nc.dram_tensor("x", [NS, F], F32, kind="ExternalInput")
    selj = nc.dram_tensor("selj", [H, NSLOT * DH], F32, kind="ExternalInput")
    seli = nc.dram_tensor("seli", [H, NSLOT], F32, kind="ExternalInput")
    out = nc.dram_tensor("out", [1, NSLOT], F32, kind="ExternalOutput")
    groups = [list(range(NC))]
    segs = _segments()
    rs_trigger = {8 * hi - 1: t for t, (lo, hi) in enumerate(RS_SLOTS)}
    WS = NSLOT * DH  # 1920

    with tile.TileContext(nc) as tc:
        with (
            tc.tile_pool(name="dram", bufs=1, space="DRAM") as dram,
            tc.tile_pool(name="xf", bufs=4) as xfp,
            tc.tile_pool(name="xb", bufs=1) as xbp,
            tc.tile_pool(name="gsb", bufs=1) as gsbp,
            tc.tile_pool(name="ps", bufs=3, space="PSUM") as psp,
            tc.tile_pool(name="sg", bufs=1) as sg,
            tc.tile_pool(name="post", bufs=2) as postp,
        ):
            # ---- DRAM internals ----
            stats_in = dram.tile([2, F], F32, tag="stats_in")
            stats_out = dram.tile([2, F], F32, tag="stats_out")
            bounce = [
                dram.tile([NC, hi - lo, DH, DH], BF16, tag=f"bounce{t}", name=f"bounce{t}")
                for t, (lo, hi) in enumerate(RS_SLOTS)
            ]
            rsout = [
                dram.tile([hi - lo, DH, DH], BF16, tag=f"rsout{t}", name=f"rsout{t}")
                for t, (lo, hi) in enumerate(RS_SLOTS)
            ]

            # ---- persistent SBUF ----
            xq = [
                xbp.tile([128, 2 * F], FP8, tag=f"xq{k}", name=f"xq{k}")
                for k in range(KCH // 2)
            ]
            xqv = [t[:].rearrange("p (two f) -> p two f", two=2) for t in xq]
            s1acc = sg.tile([128, F], F32, tag="s1acc")
            s2acc = sg.tile([128, F], F32, tag="s2acc")
            ITf = sg.tile([128, DH], F32, tag="itf")
            onesf = sg.tile([128, 1], F32, tag="ones")
            ones16 = sg.tile([16, DH], F32, tag="ones16")
            sjt = sg.tile([H, WS], F32, tag="sjt")        # selector inputs on-chip
            sit = sg.tile([H, NSLOT], F32, tag="sit")
            rT = sg.tile([H, DH], F32, tag="rT")
            qT = sg.tile([H, DH], F32, tag="qT")
            mskq = sg.tile([H, WS], F32, tag="mskq")
            RJc = sg.tile([128, WS], F32, tag="rjc")      # r_j rows by slot (bcast)
            QJc = sg.tile([128, WS], F32, tag="qjc")
            RIc = sg.tile([128, NSLOT], F32, tag="ric")   # r_i cols by slot
            QIc = sg.tile([128, NSLOT], F32, tag="qic")
            PIJ = sg.tile([128, WS], F32, tag="pij")      # r_i (x) r_j per slot
            QIJ = sg.tile([128, WS], F32, tag="qij")      # q_i (x) q_j per slot
            pl_cols = sg.tile([128, NSLOT], F32, tag="plc")

            def colt(tag, w=H):
                return sg.tile([128, w], F32, tag=tag, name=tag)

            S1c, S2c = colt("s1c"), colt("s2c")
            mu, m2, var = colt("mu"), colt("m2"), colt("var")
            sig, recip = colt("sig"), colt("recip")
            rq2 = colt("rq2", 32)          # cols 0:16 = r, 16:32 = q

            # ---- constants (gpsimd) ----
            nc.gpsimd.memset(onesf[:], 1.0)
            nc.gpsimd.memset(ones16[:], 1.0)
            make_identity(nc, ITf[:])
            nc.sync.dma_start(out=sjt[:], in_=selj[:])
            nc.sync.dma_start(out=sit[:], in_=seli[:])

            # tiny warm-up collective: absorbs the first-collective premium
            warm_in = dram.tile([1, DH], F32, tag="warm_in")
            warm_out = dram.tile([1, DH], F32, tag="warm_out")
            warmsb = sg.tile([1, DH], F32, tag="warmsb")
            nc.gpsimd.memset(warmsb[:], 1.0)
            nc.gpsimd.dma_start(out=warm_in[:], in_=warmsb[:])
            nc.gpsimd.collective_compute(
                "AllReduce",
                mybir.AluOpType.add,
                replica_groups=groups,
                ins=[warm_in[:]],
                outs=[warm_out[:]],
            )

            # ---- load, cast to bf16 (ACT), accumulate S1 (DVE) ----
            for k in range(KCH):
                xf = xfp.tile([128, F], F32, tag="xf", name="xf")
                nc.sync.dma_start(out=xf[:], in_=x[k * 128:(k + 1) * 128, :])
                nc.scalar.copy(
                    out=xq[k // 2][:, (k % 2) * F:(k % 2 + 1) * F], in_=xf[:]
                )
                sqc = postp.tile([128, F], F32, tag="sqc", name="sqc")
                nc.scalar.square(sqc[:], xf[:])
                if k == 0:
                    nc.vector.tensor_copy(out=s1acc[:], in_=xf[:])
                    nc.vector.tensor_copy(out=s2acc[:], in_=sqc[:])
                else:
                    nc.vector.tensor_add(out=s1acc[:], in0=s1acc[:], in1=xf[:])
                    nc.vector.tensor_add(out=s2acc[:], in0=s2acc[:], in1=sqc[:])

            # S1/S2 partition reduce via ones-matmul -> s1row/s2row [1, F]
            s1row = sg.tile([1, F], F32, tag="s1row")
            s2row = sg.tile([1, F], F32, tag="s2row")
            for acc, row in ((s1acc, s1row), (s2acc, s2row)):
                for t4 in range(4):
                    pss1 = psp.tile([1, 512], F32, tag="small", name="pss1", bufs=2)
                    nc.tensor.matmul(
                        pss1[0:1, :],
                        lhsT=onesf[:],
                        rhs=acc[:, t4 * 512:(t4 + 1) * 512],
                        start=True,
                        stop=True,
                    )
                    nc.vector.tensor_copy(
                        out=row[0:1, t4 * 512:(t4 + 1) * 512], in_=pss1[0:1, :]
                    )


            # stats AllReduce is emitted inside the seg loop (queue slot after RS2)
            def emit_stats_allreduce():
                nc.sync.dma_start(out=stats_in[0:1, :], in_=s1row[0:1, :])
                nc.sync.dma_start(out=stats_in[1:2, :], in_=s2row[0:1, :])
                nc.gpsimd.collective_compute(
                    "AllReduce",
                    mybir.AluOpType.add,
                    replica_groups=groups,
                    ins=[stats_in[:]],
                    outs=[stats_out[:]],
                )
                nc.sync.dma_start(
                    out=S1c[:],
                    in_=stats_out[0:1, :].rearrange("o (i d) -> o d i", i=H),
                )
                nc.sync.dma_start(
                    out=S2c[:],
                    in_=stats_out[1:2, :].rearrange("o (i d) -> o d i", i=H),
                )


            # ---- phase 1: pair-seg grams (PE) + PSUM->bf16 copies (ACT)
            #      + bounce DMAs (gpsimd) + ReduceScatters, all stats-free ----
            def emit_selector_build():
                # r/q transposes -> per-slot scale tiles, via selector matmuls.
                pst1 = psp.tile([H, DH], F32, tag="small", name="pst1", bufs=2)
                nc.tensor.transpose(pst1[:], rq2[:, 0:H], ITf[:])
                nc.vector.tensor_copy(out=rT[:], in_=pst1[:])
                pst2 = psp.tile([H, DH], F32, tag="small", name="pst2", bufs=2)
                nc.tensor.transpose(pst2[:], rq2[:, H:2 * H], ITf[:])
                nc.vector.tensor_copy(out=qT[:], in_=pst2[:])
                # masked selector rows: mskr[h, b*128+e] = selj[h,b,e] * r[h*128+e]
                sjv = sjt[:].rearrange("h (b e) -> h b e", b=NSLOT)
                nc.vector.tensor_mul(
                    out=mskq[:].rearrange("h (b e) -> h b e", b=NSLOT),
                    in0=sjv,
                    in1=qT[:].unsqueeze(1).broadcast_to([H, NSLOT, DH]),
                )
                nc.vector.tensor_mul(
                    out=sjv,
                    in0=sjv,
                    in1=rT[:].unsqueeze(1).broadcast_to([H, NSLOT, DH]),
                )
                mskr = sjt
                # RJc/QJc: broadcast gathered rows down partitions (ones x masked)
                for quarter in range(4):
                    c0, c1 = quarter * 480, (quarter + 1) * 480
                    psA = psp.tile([128, 1024], F32, tag="ps", name="psA")
                    nc.tensor.matmul(
                        psA[:, 0:480], lhsT=ones16[:], rhs=mskr[:, c0:c1],
                        start=True, stop=True,
                    )
                    nc.tensor.matmul(
                        psA[:, 512:992], lhsT=ones16[:], rhs=mskq[:, c0:c1],
                        start=True, stop=True,
                    )
                    nc.vector.tensor_copy(out=RJc[:, c0:c1], in_=psA[:, 0:480])
                    nc.vector.tensor_copy(out=QJc[:, c0:c1], in_=psA[:, 512:992])
                # RIc/QIc: per-slot r_i / q_i columns
                pst3 = psp.tile([128, NSLOT], F32, tag="small", name="pst3", bufs=2)
                nc.tensor.matmul(pst3[:], lhsT=rT[:], rhs=sit[:], start=True, stop=True)
                nc.vector.tensor_copy(out=RIc[:], in_=pst3[:])
                pst4 = psp.tile([128, NSLOT], F32, tag="small", name="pst4", bufs=2)
                nc.tensor.matmul(pst4[:], lhsT=qT[:], rhs=sit[:], start=True, stop=True)
                nc.vector.tensor_copy(out=QIc[:], in_=pst4[:])
                # fold: PIJ = RIc (x) RJc, QIJ = QIc (x) QJc (per-slot outer products)
                nc.vector.tensor_mul(
                    out=PIJ[:].rearrange("p (b e) -> p b e", b=NSLOT),
                    in0=RJc[:].rearrange("p (b e) -> p b e", b=NSLOT),
                    in1=RIc[:].unsqueeze(2).broadcast_to([128, NSLOT, DH]),
                )
                nc.vector.tensor_mul(
                    out=QIJ[:].rearrange("p (b e) -> p b e", b=NSLOT),
                    in0=QJc[:].rearrange("p (b e) -> p b e", b=NSLOT),
                    in1=QIc[:].unsqueeze(2).broadcast_to([128, NSLOT, DH]),
                )

            for si, (i, j0, nb, pbase) in enumerate(segs):
                w = nb * DH
                ps = psp.tile([128, 1024], F32, tag="ps", name="ps")
                for kp in range(KCH // 2):
                    for c0 in range(0, w, 512):
                        c1 = min(c0 + 512, w)
                        nc.tensor.matmul(
                            ps[:, c0:c1],
                            lhsT=xqv[kp][:, :, i * DH:(i + 1) * DH],
                            rhs=xqv[kp][:, :, j0 * DH + c0:j0 * DH + c1],
                            start=(kp == 0),
                            stop=(kp == KCH // 2 - 1),
                            perf_mode=mybir.MatmulPerfMode.DoubleRow,
                        )
                gs = gsbp.tile([128, w], BF16, tag=f"gs{si}", name=f"gs{si}", bufs=1)
                nc.scalar.copy(out=gs[:], in_=ps[:, :w])
                # grouped bounce DMAs: runs of consecutive p share the slot b
                p = pbase
                while p < pbase + nb:
                    b, c0 = p // 8, p % 8
                    ln = min(8 - c0, pbase + nb - p)
                    t = next(tt for tt, (lo, hi) in enumerate(RS_SLOTS) if lo <= b < hi)
                    lo, hi = RS_SLOTS[t]
                    m0 = p - pbase
                    src = gs[:, m0 * DH:(m0 + ln) * DH].rearrange(
                        "z (m e) -> z m e", m=ln
                    )
                    dst = bounce[t][c0:c0 + ln, b - lo, :, :].rearrange(
                        "c d e -> d c e"
                    )
                    nc.gpsimd.dma_start(out=dst, in_=src)
                    p += ln
                for p in range(pbase, pbase + nb):
                    if p in rs_trigger:
                        t = rs_trigger[p]
                        nc.gpsimd.collective_compute(
                            "ReduceScatter",
                            mybir.AluOpType.add,
                            replica_groups=groups,
                            ins=[bounce[t][:]],
                            outs=[rsout[t][:]],
                        )
                        if t == 0:
                            emit_stats_allreduce()

            # ---- stats math in [128(d), 16(i)] layout (DVE; sqrt on ACT) ----
            nc.vector.tensor_scalar_mul(mu[:], S1c[:], 1.0 / N)
            nc.vector.tensor_mul(out=m2[:], in0=mu[:], in1=mu[:])
            nc.vector.tensor_scalar_mul(m2[:], m2[:], -float(N))
            nc.vector.tensor_add(out=var[:], in0=S2c[:], in1=m2[:])
            nc.vector.tensor_scalar_mul(var[:], var[:], 1.0 / (N - 1))
            nc.scalar.sqrt(sig[:], var[:])
            nc.vector.tensor_scalar_add(sig[:], sig[:], EPS)
            nc.vector.reciprocal(recip[:], sig[:])                 # 1/(sigma+eps)
            nc.vector.tensor_scalar_mul(
                rq2[:, 0:H], recip[:], 1.0 / math.sqrt(N)
            )                                                      # r
            nc.vector.tensor_mul(out=rq2[:, H:2 * H], in0=mu[:], in1=recip[:])  # q

            emit_selector_build()

            # ---- phase 2: post-RS standardization + pair losses (15 blocks) ----
            for t, (lo, hi) in enumerate(RS_SLOTS):
                nb = hi - lo
                wc = nb * DH
                rbC = postp.tile([128, 5 * DH], BF16, tag="rbC", name="rbC")
                nc.sync.dma_start(
                    out=rbC[:, :wc].rearrange("d (m e) -> d m e", m=nb),
                    in_=rsout[t][:].rearrange("b d e -> d b e"),
                )
                u32 = postp.tile([128, 5 * DH], F32, tag="u32", name="u32")
                nc.vector.tensor_mul(
                    out=u32[:, :wc], in0=rbC[:, :wc],
                    in1=PIJ[:, lo * DH:lo * DH + wc],
                )
                nc.vector.tensor_sub(
                    out=u32[:, :wc], in0=u32[:, :wc],
                    in1=QIJ[:, lo * DH:lo * DH + wc],
                )
                nc.vector.tensor_sub(
                    out=u32[:, :wc].rearrange("d (m e) -> d m e", m=nb),
                    in0=u32[:, :wc].rearrange("d (m e) -> d m e", m=nb),
                    in1=ITf[:].unsqueeze(1).broadcast_to([128, nb, DH]),
                )
                for b in range(nb):
                    nc.scalar.activation(
                        out=u32[:, b * DH:(b + 1) * DH],
                        in_=u32[:, b * DH:(b + 1) * DH],
                        func=mybir.ActivationFunctionType.Square,
                        accum_out=pl_cols[:, lo + b:lo + b + 1],
                    )

            # partition-reduce pair losses and write out
            pspl = psp.tile([128, 1024], F32, tag="ps", name="pspl")
            nc.tensor.matmul(
                pspl[0:1, 0:NSLOT], lhsT=onesf[:], rhs=pl_cols[:], start=True, stop=True
            )
            outsb = sg.tile([1, NSLOT], F32, tag="outsb")
            nc.vector.tensor_copy(out=outsb[:], in_=pspl[0:1, 0:NSLOT])
            nc.sync.dma_start(out=out[:], in_=outsb[:])

    nc.compile()
    return nc


_NC_CACHE = None


def _get_nc():
    global _NC_CACHE
    if _NC_CACHE is None:
        _NC_CACHE = build()
    return _NC_CACHE


def _selectors():
    """Per-core 0/1 selector tensors for the post-RS gather matmuls."""
    sel = []
    for c in range(NC):
        sj = np.zeros((H, NSLOT * DH), np.float32)
        si = np.zeros((H, NSLOT), np.float32)
        for b in range(NSLOT):
            i, j = PAIRS[8 * b + c]
            sj[j, b * DH:(b + 1) * DH] = 1.0
            si[i, b] = 1.0
        sel.append((sj, si))
    return sel


_SEL = _selectors()


def _make_in_maps(head_outputs):
    shards = np.asarray(head_outputs, dtype=np.float32).reshape(NC, NS, F)
    return [
        {"x": np.ascontiguousarray(shards[c]), "selj": _SEL[c][0], "seli": _SEL[c][1]}
        for c in range(NC)
    ]


def _combine(results, G):
    pl = np.zeros(len(PAIRS), np.float64)
    for c in range(NC):
        o = np.asarray(results[c]["out"], dtype=np.float64).reshape(NSLOT)
        for b in range(NSLOT):
            pl[8 * b + c] = o[b]
    Gd = np.asarray(G, dtype=np.float64)
    w = ALPHA + (1.0 - ALPHA) * np.logaddexp(0.0, -BETA * (Gd - TAU))
    loss = sum(w[i, j] * pl[p] for p, (i, j) in enumerate(PAIRS)) / len(PAIRS)
    return np.asarray(loss, dtype=np.float32)


def kernel(head_outputs, G):
    nc = _get_nc()
    res = run_bass_kernel_spmd(nc, _make_in_maps(head_outputs), list(range(NC)))
    return _combine(res.results, G)


def timed_run(head_outputs, G, **kw):
    """Run with NTFF profiling; returns (loss, BassKernelResults)."""
    nc = _get_nc()
    res = run_bass_kernel_spmd(
        nc, _make_in_maps(head_outputs), list(range(NC)), trace=True, **kw
    )
    return _combine(res.results, G), res

